# revision 1
# baseline (speedup 1.0000x reference)
"""LocalMerge kernel for 8 trn2 NeuronCores.

v0 strategy: data-parallel execution of the full module across the 8
axon-attached NeuronCores via jax/XLA-Neuron. Batch (4) x branch-internal
query split supplies the parallelism; BatchNorm statistics are handled by
XLA's partitioner (all-reduces over the device mesh).
"""

import numpy as np

KNN = 32
B, N, CIN, COUT = 4, 2048, 128, 256


def _reference_impl(jnp, jax):
    def _sqdist(src, dst):
        return (jnp.sum(src * src, -1)[:, :, None]
                + jnp.sum(dst * dst, -1)[:, None, :]
                - 2.0 * jnp.einsum('bnc,bmc->bnm', src, dst))

    def _knn(points, queries, k):
        d = _sqdist(queries, points)
        neg_d, idx = jax.lax.top_k(-d, k)
        return -neg_d, idx

    def _gather(points, idx):
        return jax.vmap(lambda p, i: p[i])(points, idx)

    def _bn(x, gamma, beta):
        mean = jnp.mean(x, axis=(0, 1), keepdims=True)
        var = jnp.var(x, axis=(0, 1), keepdims=True)
        return gamma * (x - mean) * jax.lax.rsqrt(var + 1e-5) + beta

    def _lin_bn_act(x, W, b, gamma, beta):
        return jax.nn.leaky_relu(_bn(x @ W + b, gamma, beta), 0.2)

    def _local_trans(feat, idx, qW, qb, kW, kb, vW, vb, rW, rb, rg, rbe,
                     fW, fb, fg, fbe):
        residual = _lin_bn_act(feat, rW, rb, rg, rbe)
        q = (feat @ qW + qb)[:, :, None, :]
        k = _gather(feat @ kW + kb, idx)
        v = _gather(feat @ vW + vb, idx)
        energy = (q - k) / jnp.sqrt(jnp.float32(qW.shape[-1]))
        att = jax.nn.softmax(energy, axis=2)
        att = att - jnp.sum(att, axis=2, keepdims=True)
        ctx = jnp.max(att * v, axis=2)
        return residual + _lin_bn_act(ctx, fW, fb, fg, fbe)

    def forward(xyz, base_xyz, feature, qW, qb, kW, kb, vW, vb, resW, resb,
                res_gamma, res_beta, ffnW, ffnb, ffn_gamma, ffn_beta,
                fcW, fcb, fc_gamma, fc_beta):
        dist, idx = _knn(base_xyz, xyz, KNN)
        _, idx_feature = _knn(feature, feature, KNN)
        idxs = (idx, idx_feature)
        ms = [_local_trans(feature, idxs[i], qW[i], qb[i], kW[i], kb[i],
                           vW[i], vb[i], resW[i], resb[i], res_gamma[i],
                           res_beta[i], ffnW[i], ffnb[i], ffn_gamma[i],
                           ffn_beta[i]) for i in range(2)]
        merge = _lin_bn_act(jnp.concatenate(ms, axis=2), fcW, fcb,
                            fc_gamma, fc_beta)
        return merge

    return forward


_JITTED = {}


def _get_jitted():
    if "fn" in _JITTED:
        return _JITTED["fn"]
    import jax
    import jax.numpy as jnp
    from jax.sharding import Mesh, PartitionSpec as P, NamedSharding

    devs = jax.devices()[:8]
    mesh = Mesh(np.array(devs).reshape(4, 2), ("b", "n"))
    fwd = _reference_impl(jnp, jax)

    pt3 = NamedSharding(mesh, P("b", "n", None))   # [B, N, *]
    rep = NamedSharding(mesh, P())                 # replicated params

    in_shardings = (pt3, pt3, pt3) + (rep,) * 18
    out_sharding = NamedSharding(mesh, P("b", "n", None))

    fn = jax.jit(fwd, in_shardings=in_shardings, out_shardings=out_sharding)
    _JITTED["fn"] = (fn, jax, jnp)
    return _JITTED["fn"]


_ARG_ORDER = ["xyz", "base_xyz", "feature", "qW", "qb", "kW", "kb", "vW",
              "vb", "resW", "resb", "res_gamma", "res_beta", "ffnW", "ffnb",
              "ffn_gamma", "ffn_beta", "fcW", "fcb", "fc_gamma", "fc_beta"]


def kernel(**inputs) -> np.ndarray:
    fn, jax, jnp = _get_jitted()
    args = [np.asarray(inputs[k]) for k in _ARG_ORDER]
    out = fn(*args)
    return np.asarray(out).astype(np.float32)


if __name__ == "__main__":
    import reference
    ins = {k: np.asarray(v) for k, v in reference.setup_inputs().items()}
    out = kernel(**ins)
    print(out.shape, out.dtype)
